# revision 4
# baseline (speedup 1.0000x reference)
"""Trainium2 Bass kernel for the fused cross-attention layer.

Math restructuring (exact):
    S = Q_a K_a^T + (Q_a M_av^T)^T
      = a (W_q^T W_k) a^T + (a+v) (W_m^T W_q) a^T
      = B a^T,   B = a G + (a+v) G2,  G = W_q^T W_k,  G2 = W_m^T W_q
    alpha = softmax(S, axis=1);  out = alpha @ (a W_v^T);  feat = out + a

So the N x N score matrix is a single [N,H]x[H,N] matmul instead of two,
and the alpha_av^T "all-to-all coupling" disappears: each core only needs
its local rows of B plus the shared a^T stream.

Sharding: rows of the score matrix across 8 cores (1024 rows each).
Each core streams all 64 column-chunks of a^T (host-pretransposed, so
the PE does no transposes in the main loop), computes S^T (columns on
partitions) so the softmax-weighted PV matmul needs no P transposes,
and accumulates output in PSUM/SBUF.  G/G2/W_v^T are tiny host-side
weight preprocessing; P and the PV matmul run in bf16 (tolerance is
2e-2), with the softmax denominators produced by N=1 matmuls that share
the PV matmul's stationary operand.

Softmax shift: exact row-max is a partition-dim reduction in this layout,
but softmax only needs any per-row shift within ~+-80 of the true max.
We use m_r = 4.2*||B_r|| + 25 (validated: m - rowmax in [-14, +38] here,
overflow would need a >12-sigma row maximum).

All heavy fp32 matmuls run as float32r (FP22 mantissa truncation, full
PE rate); accumulation is always fp32 in PSUM.
"""

import sys

sys.path.insert(0, "/opt/trn_rl_repo")

from contextlib import ExitStack

import ml_dtypes
import numpy as np

import concourse.bacc as bacc
import concourse.bass as bass
import concourse.mybir as mybir
import concourse.tile as tile
from concourse.bass_utils import run_bass_kernel_spmd
from concourse.masks import make_identity

N, H, NCORE = 8192, 512, 8
R = N // NCORE          # 1024 rows per core
RC = R // 128           # 8 row chunks per core
FC = H // 128           # 4 feature chunks
CC = N // 128           # 64 column chunks (full N)
GRP = 8                 # column chunks per group
NG = CC // GRP

F32 = mybir.dt.float32
F32R = mybir.dt.float32r
BF16 = mybir.dt.bfloat16

MAX_SCALE = 4.2         # m_r = MAX_SCALE * ||B_r|| + MAX_BIAS
MAX_BIAS = 25.0


def build():
    nc = bacc.Bacc("TRN2", target_bir_lowering=False, debug=False,
                   num_devices=NCORE)
    aT = nc.dram_tensor("aT", [H, N], F32, kind="ExternalInput").ap()
    ab = nc.dram_tensor("ab", [N, H], BF16, kind="ExternalInput").ap()
    aT_loc = nc.dram_tensor("aT_loc", [H, R], F32, kind="ExternalInput").ap()
    avT_loc = nc.dram_tensor("avT_loc", [H, R], F32, kind="ExternalInput").ap()
    a_loc = nc.dram_tensor("a_loc", [R, H], F32, kind="ExternalInput").ap()
    g1 = nc.dram_tensor("g1", [H, H], F32, kind="ExternalInput").ap()
    g2 = nc.dram_tensor("g2", [H, H], F32, kind="ExternalInput").ap()
    wvT = nc.dram_tensor("wvT", [H, H], F32, kind="ExternalInput").ap()
    out_att = nc.dram_tensor("out_att", [R, H], F32, kind="ExternalOutput").ap()
    feat = nc.dram_tensor("feat", [R, H], F32, kind="ExternalOutput").ap()

    with tile.TileContext(nc) as tc, ExitStack() as ctx:
        persist = ctx.enter_context(tc.tile_pool(name="persist", bufs=1))
        id_s = persist.tile([128, 128], F32)
        make_identity(nc, id_s)
        ones_s = persist.tile([128, 1], F32R)
        ones_dram = nc.inline_tensor(np.ones((128, 1), np.float32), "ones_c")
        nc.sync.dma_start(out=ones_s, in_=ones_dram.ap().bitcast(F32R))
        ones_b = persist.tile([128, 1], BF16)
        nc.vector.memset(ones_b, 1.0)
        ones_row = persist.tile([1, 128], F32)
        nc.vector.memset(ones_row, 1.0)
        wvT_s = persist.tile([128, FC, H], F32R)    # W_v^T: [h, h']
        nc.sync.dma_start(
            out=wvT_s,
            in_=wvT.rearrange("(c p) n -> p c n", p=128).bitcast(F32R))
        BT_s = persist.tile([128, FC, R], F32R)     # B^T local: [f, r]
        out_acc = persist.tile([128, RC, H], F32)   # PV accumulator
        negm_bc = persist.tile([128, R], F32)       # -m_r broadcast to 128 p

        # ----------------- setup -----------------
        with ExitStack() as sctx:
            sp = sctx.enter_context(tc.tile_pool(name="setup", bufs=1))
            spp = sctx.enter_context(
                tc.tile_pool(name="setup_ps", bufs=2, space="PSUM"))

            # local a^T and (a+v)^T  (host-pretransposed)
            aTl = sp.tile([128, FC, R], F32R)
            nc.sync.dma_start(
                out=aTl,
                in_=aT_loc.rearrange("(c p) n -> p c n", p=128).bitcast(F32R))
            avTl = sp.tile([128, FC, R], F32R)
            nc.sync.dma_start(
                out=avTl,
                in_=avT_loc.rearrange("(c p) n -> p c n", p=128).bitcast(F32R))
            g1_s = sp.tile([128, FC, H], F32R)
            nc.sync.dma_start(
                out=g1_s,
                in_=g1.rearrange("(c p) n -> p c n", p=128).bitcast(F32R))
            g2_s = sp.tile([128, FC, H], F32R)
            nc.sync.dma_start(
                out=g2_s,
                in_=g2.rearrange("(c p) n -> p c n", p=128).bitcast(F32R))

            # B^T = G^T a^T + G2^T av^T  ([f, r], f on partitions)
            for fc in range(FC):
                for rt in range(R // 512):
                    ps_b = spp.tile([128, 512], F32, name="ps_gen")
                    tsl = slice(512 * rt, 512 * rt + 512)
                    msl = slice(128 * fc, 128 * fc + 128)
                    for kc in range(FC):
                        nc.tensor.matmul(ps_b, (g1_s[:, kc, msl]),
                                         (aTl[:, kc, tsl]),
                                         start=(kc == 0), stop=False)
                    for kc in range(FC):
                        nc.tensor.matmul(ps_b, (g2_s[:, kc, msl]),
                                         (avTl[:, kc, tsl]),
                                         start=False, stop=(kc == FC - 1))
                    nc.vector.tensor_copy(BT_s[:, fc, tsl], ps_b)

            # -m_r = -(MAX_SCALE * ||B_r|| + MAX_BIAS), broadcast to 128 p
            bt2 = sp.tile([128, FC, R], F32R)
            for fc in range(FC):
                nc.scalar.square(bt2[:, fc, :], BT_s[:, fc, :])
            negm_f32 = sp.tile([1, R], F32)
            for rt in range(R // 512):
                tsl = slice(512 * rt, 512 * rt + 512)
                ps_n = spp.tile([1, 512], F32, name="ps_n")
                for fc in range(FC):
                    nc.tensor.matmul(ps_n, (ones_s), (bt2[:, fc, tsl]),
                                     start=(fc == 0), stop=(fc == FC - 1))
                nc.scalar.sqrt(negm_f32[0:1, tsl], ps_n)
            nc.vector.tensor_scalar(out=negm_f32, in0=negm_f32,
                                    scalar1=-MAX_SCALE, scalar2=-MAX_BIAS,
                                    op0=mybir.AluOpType.mult,
                                    op1=mybir.AluOpType.add)
            for rt in range(R // 512):
                tsl = slice(512 * rt, 512 * rt + 512)
                ps_bc = spp.tile([128, 512], F32, name="ps_gen")
                nc.tensor.matmul(ps_bc, ones_row, negm_f32[0:1, tsl],
                                 start=True, stop=True)
                nc.vector.tensor_copy(negm_bc[:, tsl], ps_bc)

        # ----------------- main sweep -----------------
        pl_ps = ctx.enter_context(
            tc.tile_pool(name="ps_l", bufs=1, space="PSUM"))
        psl = pl_ps.tile([128, RC], F32)            # row sums l, [r, rc]
        with ExitStack() as mctx:
            ap_pool = mctx.enter_context(tc.tile_pool(name="atiles", bufs=2))
            pp = mctx.enter_context(tc.tile_pool(name="ptiles", bufs=GRP + 2))
            ps_ps = mctx.enter_context(
                tc.tile_pool(name="ps_s", bufs=3, space="PSUM"))
            po_ps = mctx.enter_context(
                tc.tile_pool(name="ps_o", bufs=2, space="PSUM"))
            aT_r = aT.rearrange("(c p) n -> p c n", p=128).bitcast(F32R)
            ab_r = ab.rearrange("(c p) n -> p c n", p=128)
            for g in range(NG):
                csl = slice(128 * GRP * g, 128 * GRP * (g + 1))
                aTg = ap_pool.tile([128, FC, 128 * GRP], F32R, name="aTg")
                nc.sync.dma_start(out=aTg, in_=aT_r[:, :, csl])
                abg = ap_pool.tile([128, GRP, H], BF16, name="abg")
                nc.sync.dma_start(
                    out=abg, in_=ab_r[:, GRP * g:GRP * (g + 1), :])

                pts = []
                for j in range(GRP):
                    jsl = slice(128 * j, 128 * j + 128)
                    PT = pp.tile([128, R], BF16, name="PT")
                    for rt in range(R // 512):
                        tsl = slice(512 * rt, 512 * rt + 512)
                        ps_s = ps_ps.tile([128, 512], F32, name="ps_s")
                        for fc in range(FC):
                            nc.tensor.matmul(ps_s, (aTg[:, fc, jsl]),
                                             (BT_s[:, fc, tsl]),
                                             start=(fc == 0),
                                             stop=(fc == FC - 1))
                        nc.vector.tensor_add(ps_s, ps_s, negm_bc[:, tsl])
                        nc.scalar.activation(PT[:, tsl], ps_s,
                                             func=mybir.ActivationFunctionType.Exp)
                    pts.append(PT)

                for rc in range(RC):
                    ps_o = po_ps.tile([128, H], F32, name="ps_o")
                    rsl = slice(128 * rc, 128 * rc + 128)
                    for j in range(GRP):
                        nc.tensor.matmul(ps_o, (pts[j][:, rsl]), (abg[:, j, :]),
                                         start=(j == 0), stop=(j == GRP - 1))
                        # row-sum l via N=1 matmul, same stationary operand.
                        # One accumulation group for the whole bank: only the
                        # very first matmul clears has_written; each column's
                        # first touch then overwrites, later ones accumulate.
                        nc.tensor.matmul(psl[:, rc:rc + 1], (pts[j][:, rsl]),
                                         (ones_b),
                                         start=(g == 0 and rc == 0 and j == 0),
                                         stop=(g == NG - 1 and rc == RC - 1
                                               and j == GRP - 1))
                    if g == 0:
                        nc.vector.tensor_copy(out_acc[:, rc, :], ps_o)
                    else:
                        nc.vector.tensor_add(out_acc[:, rc, :],
                                             out_acc[:, rc, :], ps_o)

        # ----------------- epilogue -----------------
        with ExitStack() as ectx:
            ep = ectx.enter_context(tc.tile_pool(name="epil", bufs=4))
            ept_ps = ectx.enter_context(
                tc.tile_pool(name="ps_ept", bufs=2, space="PSUM"))
            epo_ps = ectx.enter_context(
                tc.tile_pool(name="ps_epo", bufs=2, space="PSUM"))
            rinv = ep.tile([128, RC], F32)
            nc.vector.reciprocal(rinv, psl)
            for rc in range(RC):
                rsl = slice(128 * rc, 128 * rc + 128)
                # transpose PA chunk: out_acc[:, rc, :] -> [f, r]
                ps_pt = ept_ps.tile([128, H], F32, name="ps_pt")
                for fc in range(FC):
                    fsl = slice(128 * fc, 128 * fc + 128)
                    nc.tensor.transpose(ps_pt[:, fsl], out_acc[:, rc, fsl],
                                        id_s)
                pat = ep.tile([128, FC, 128], F32R, name="pat")
                nc.scalar.copy(pat,
                               ps_pt.rearrange("p (c j) -> p c j", j=128))
                # att = (PA @ WvT) / l
                ps_att = epo_ps.tile([128, H], F32, name="ps_att")
                for fc in range(FC):
                    nc.tensor.matmul(ps_att, pat[:, fc, :], wvT_s[:, fc, :],
                                     start=(fc == 0), stop=(fc == FC - 1))
                att = ep.tile([128, H], F32, name="att")
                nc.vector.tensor_scalar_mul(att, ps_att, rinv[:, rc:rc + 1])
                nc.sync.dma_start(out=out_att[rsl, :], in_=att)
                al2 = ep.tile([128, H], F32, name="al2")
                nc.sync.dma_start(out=al2, in_=a_loc[rsl, :])
                ft = ep.tile([128, H], F32, name="ft")
                nc.vector.tensor_add(ft, att, al2)
                nc.sync.dma_start(out=feat[rsl, :], in_=ft)

    nc.finalize()
    return nc


_NC_CACHE = []


def _get_nc():
    if not _NC_CACHE:
        _NC_CACHE.append(build())
    return _NC_CACHE[0]


def make_in_maps(inputs_a, inputs_v, W_q, W_k, W_v, W_m):
    a = np.ascontiguousarray(np.asarray(inputs_a, dtype=np.float32))
    v = np.ascontiguousarray(np.asarray(inputs_v, dtype=np.float32))
    wq = np.asarray(W_q, dtype=np.float32)
    wk = np.asarray(W_k, dtype=np.float32)
    wv = np.asarray(W_v, dtype=np.float32)
    wm = np.asarray(W_m, dtype=np.float32)
    aT = np.ascontiguousarray(a.T)
    ab = np.ascontiguousarray(a.astype(ml_dtypes.bfloat16))
    g1v = np.ascontiguousarray(wq.T @ wk)
    g2v = np.ascontiguousarray(wm.T @ wq)
    wvTv = np.ascontiguousarray(wv.T)
    in_maps = []
    for i in range(NCORE):
        sl = slice(R * i, R * (i + 1))
        in_maps.append({
            "aT": aT,
            "ab": ab,
            "aT_loc": np.ascontiguousarray(aT[:, sl]),
            "avT_loc": np.ascontiguousarray((a[sl] + v[sl]).T),
            "a_loc": np.ascontiguousarray(a[sl]),
            "g1": g1v,
            "g2": g2v,
            "wvT": wvTv,
        })
    return in_maps


def kernel(inputs_a, inputs_v, W_q, W_k, W_v, W_m, _run_kwargs=None):
    nc = _get_nc()
    in_maps = make_in_maps(inputs_a, inputs_v, W_q, W_k, W_v, W_m)
    res = run_bass_kernel_spmd(nc, in_maps, list(range(NCORE)),
                               **(_run_kwargs or {}))
    out_attention = np.concatenate(
        [res.results[i]["out_att"] for i in range(NCORE)], axis=0)
    feature_map = np.concatenate(
        [res.results[i]["feat"] for i in range(NCORE)], axis=0)
    kernel.last_results = res
    return (out_attention, feature_map)


# revision 9
# speedup vs baseline: 1.0825x; 1.0825x over previous
"""Trainium2 Bass kernel for the fused cross-attention layer.

Math restructuring (exact):
    S = Q_a K_a^T + (Q_a M_av^T)^T
      = a (W_q^T W_k) a^T + (a+v) (W_m^T W_q) a^T
      = B a^T,   B = a G + (a+v) G2,  G = W_q^T W_k,  G2 = W_m^T W_q
    alpha = softmax(S, axis=1);  out = alpha @ (a W_v^T);  feat = out + a

So the N x N score matrix is a single [N,H]x[H,N] matmul instead of two,
and the alpha_av^T "all-to-all coupling" disappears: each core only needs
its local rows of B plus the shared a^T stream.

Sharding: rows of the score matrix across 8 cores (1024 rows each).
Each core streams all 64 column-chunks of a^T (host-pretransposed, so
the PE does no transposes in the main loop), computes S^T (columns on
partitions) so the softmax-weighted PV matmul needs no P transposes,
and accumulates output in PSUM/SBUF.  G/G2/W_v^T are tiny host-side
weight preprocessing; P and the PV matmul run in bf16 (tolerance is
2e-2), with the softmax denominators produced by N=1 matmuls that share
the PV matmul's stationary operand.

Softmax shift: exact row-max is a partition-dim reduction in this layout,
but softmax only needs any per-row shift within ~+-80 of the true max.
We use m_r = 4.2*||B_r|| + 25 (validated: m - rowmax in [-14, +38] here,
overflow would need a >12-sigma row maximum).

All heavy fp32 matmuls run as float32r (FP22 mantissa truncation, full
PE rate); accumulation is always fp32 in PSUM.
"""

import sys

sys.path.insert(0, "/opt/trn_rl_repo")

from contextlib import ExitStack

import ml_dtypes
import numpy as np

import concourse.bacc as bacc
import concourse.bass as bass
import concourse.mybir as mybir
import concourse.tile as tile
from concourse.bass_utils import run_bass_kernel_spmd
from concourse.masks import make_identity

N, H, NCORE = 8192, 512, 8
R = N // NCORE          # 1024 rows per core
RC = R // 128           # 8 row chunks per core
FC = H // 128           # 4 feature chunks
CC = N // 128           # 64 column chunks (full N)
GRP = 8                 # column chunks per group
NG = CC // GRP

F32 = mybir.dt.float32
F32R = mybir.dt.float32r
BF16 = mybir.dt.bfloat16

MAX_SCALE = 4.2         # m_r = MAX_SCALE * ||B_r|| + MAX_BIAS
MAX_BIAS = 25.0


def build():
    nc = bacc.Bacc("TRN2", target_bir_lowering=False, debug=False,
                   num_devices=NCORE)
    aT = nc.dram_tensor("aT", [H, N], F32, kind="ExternalInput").ap()
    ab = nc.dram_tensor("ab", [N, H], BF16, kind="ExternalInput").ap()
    aT_loc = nc.dram_tensor("aT_loc", [H, R], F32, kind="ExternalInput").ap()
    avT_loc = nc.dram_tensor("avT_loc", [H, R], F32, kind="ExternalInput").ap()
    a_loc = nc.dram_tensor("a_loc", [R, H], F32, kind="ExternalInput").ap()
    g1 = nc.dram_tensor("g1", [H, H], F32, kind="ExternalInput").ap()
    g2 = nc.dram_tensor("g2", [H, H], F32, kind="ExternalInput").ap()
    wvT = nc.dram_tensor("wvT", [H, H], F32, kind="ExternalInput").ap()
    out_att = nc.dram_tensor("out_att", [R, H], F32, kind="ExternalOutput").ap()
    feat = nc.dram_tensor("feat", [R, H], F32, kind="ExternalOutput").ap()

    with tile.TileContext(nc) as tc, ExitStack() as ctx:
        persist = ctx.enter_context(tc.tile_pool(name="persist", bufs=1))
        id_s = persist.tile([128, 128], F32)
        make_identity(nc, id_s)
        ones_s = persist.tile([128, 1], F32R)
        ones_dram = nc.inline_tensor(np.ones((128, 1), np.float32), "ones_c")
        nc.sync.dma_start(out=ones_s, in_=ones_dram.ap().bitcast(F32R))
        ones_b = persist.tile([128, 1], BF16)
        nc.vector.memset(ones_b, 1.0)
        ones_row = persist.tile([1, 128], F32)
        nc.vector.memset(ones_row, 1.0)
        wvT_s = persist.tile([128, FC, H], F32R)    # W_v^T: [h, h']
        BT_s = persist.tile([128, FC, R], F32R)     # B^T local: [f, r]
        out_acc = persist.tile([128, RC, H], F32)   # PV accumulator
        negm_bc = persist.tile([128, R], F32)       # -m_r broadcast to 128 p
        a_loc_s = persist.tile([128, RC, H], F32)   # local a rows (epilogue)

        aT_r = aT.rearrange("(c p) n -> p c n", p=128).bitcast(F32R)
        ab_r = ab.rearrange("(c p) n -> p c n", p=128)
        ap_pool = ctx.enter_context(tc.tile_pool(name="atiles", bufs=2))

        # ----------------- setup -----------------
        with ExitStack() as sctx:
            sp = sctx.enter_context(tc.tile_pool(name="setup", bufs=1))

            # DMA issue order = criticality: the B^T matmuls need
            # aTl+g1 first, then avTl+g2; prefetch group 0's stream
            # behind those; epilogue-only tensors last.
            aTl = sp.tile([128, FC, R], F32R)
            nc.sync.dma_start(
                out=aTl,
                in_=aT_loc.rearrange("(c p) n -> p c n", p=128).bitcast(F32R))
            g1_s = sp.tile([128, FC, H], F32R)
            nc.sync.dma_start(
                out=g1_s,
                in_=g1.rearrange("(c p) n -> p c n", p=128).bitcast(F32R))
            avTl = sp.tile([128, FC, R], F32R)
            nc.sync.dma_start(
                out=avTl,
                in_=avT_loc.rearrange("(c p) n -> p c n", p=128).bitcast(F32R))
            g2_s = sp.tile([128, FC, H], F32R)
            nc.sync.dma_start(
                out=g2_s,
                in_=g2.rearrange("(c p) n -> p c n", p=128).bitcast(F32R))
            aTg0 = ap_pool.tile([128, FC, 128 * GRP], F32R, name="aTg")
            nc.sync.dma_start(out=aTg0, in_=aT_r[:, :, 0:128 * GRP])
            abg0 = ap_pool.tile([128, GRP, H], BF16, name="abg")
            nc.sync.dma_start(out=abg0, in_=ab_r[:, 0:GRP, :])
            nc.sync.dma_start(
                out=a_loc_s,
                in_=a_loc.rearrange("(c p) n -> p c n", p=128))
            nc.sync.dma_start(
                out=wvT_s,
                in_=wvT.rearrange("(c p) n -> p c n", p=128).bitcast(F32R))

            # B^T = G^T a^T + G2^T av^T  ([f, r], f on partitions),
            # two passes over 8 PSUM banks so the G pass starts as soon
            # as aTl+g1 land (avTl+g2 are still in flight).
            with ExitStack() as btctx:
                btp = btctx.enter_context(
                    tc.tile_pool(name="bt_ps", bufs=8, space="PSUM"))
                bt_ps = []
                for fc in range(FC):
                    for rt in range(R // 512):
                        ps_b = btp.tile([128, 512], F32, name="ps_bt")
                        msl = slice(128 * fc, 128 * fc + 128)
                        tsl = slice(512 * rt, 512 * rt + 512)
                        for kc in range(FC):
                            nc.tensor.matmul(ps_b, (g1_s[:, kc, msl]),
                                             (aTl[:, kc, tsl]),
                                             start=(kc == 0), stop=False)
                        bt_ps.append(ps_b)
                for fc in range(FC):
                    for rt in range(R // 512):
                        ps_b = bt_ps[fc * (R // 512) + rt]
                        msl = slice(128 * fc, 128 * fc + 128)
                        tsl = slice(512 * rt, 512 * rt + 512)
                        for kc in range(FC):
                            nc.tensor.matmul(ps_b, (g2_s[:, kc, msl]),
                                             (avTl[:, kc, tsl]),
                                             start=False, stop=(kc == FC - 1))
                        nc.vector.tensor_copy(BT_s[:, fc, tsl], ps_b)
            spp = sctx.enter_context(
                tc.tile_pool(name="setup_ps", bufs=2, space="PSUM"))

            # -m_r = -(MAX_SCALE * ||B_r|| + MAX_BIAS), broadcast to 128 p
            bt2 = sp.tile([128, FC, R], F32R)
            for fc in range(FC):
                nc.scalar.square(bt2[:, fc, :], BT_s[:, fc, :])
            negm_f32 = sp.tile([1, R], F32)
            for rt in range(R // 512):
                tsl = slice(512 * rt, 512 * rt + 512)
                ps_n = spp.tile([1, 512], F32, name="ps_n")
                for fc in range(FC):
                    nc.tensor.matmul(ps_n, (ones_s), (bt2[:, fc, tsl]),
                                     start=(fc == 0), stop=(fc == FC - 1))
                nc.scalar.sqrt(negm_f32[0:1, tsl], ps_n)
            nc.vector.tensor_scalar(out=negm_f32, in0=negm_f32,
                                    scalar1=-MAX_SCALE, scalar2=-MAX_BIAS,
                                    op0=mybir.AluOpType.mult,
                                    op1=mybir.AluOpType.add)
            for rt in range(R // 512):
                tsl = slice(512 * rt, 512 * rt + 512)
                ps_bc = spp.tile([128, 512], F32, name="ps_gen")
                nc.tensor.matmul(ps_bc, ones_row, negm_f32[0:1, tsl],
                                 start=True, stop=True)
                nc.vector.tensor_copy(negm_bc[:, tsl], ps_bc)

        # ----------------- main sweep -----------------
        pl_ps = ctx.enter_context(
            tc.tile_pool(name="ps_l", bufs=1, space="PSUM"))
        psl = pl_ps.tile([128, RC], F32)            # row sums l, [r, rc]
        with ExitStack() as mctx:
            pp = mctx.enter_context(tc.tile_pool(name="ptiles", bufs=GRP + 2))
            ps_ps = mctx.enter_context(
                tc.tile_pool(name="ps_s", bufs=4, space="PSUM"))
            po_ps = mctx.enter_context(
                tc.tile_pool(name="ps_o", bufs=2, space="PSUM"))
            for g in range(NG):
                if g == 0:
                    aTg, abg = aTg0, abg0
                else:
                    csl = slice(128 * GRP * g, 128 * GRP * (g + 1))
                    aTg = ap_pool.tile([128, FC, 128 * GRP], F32R, name="aTg")
                    nc.sync.dma_start(out=aTg, in_=aT_r[:, :, csl])
                    abg = ap_pool.tile([128, GRP, H], BF16, name="abg")
                    nc.sync.dma_start(
                        out=abg, in_=ab_r[:, GRP * g:GRP * (g + 1), :])

                pts = []
                for j in range(GRP):
                    jsl = slice(128 * j, 128 * j + 128)
                    PT = pp.tile([128, R], BF16, name="PT")
                    for rt in range(R // 512):
                        tsl = slice(512 * rt, 512 * rt + 512)
                        ps_s = ps_ps.tile([128, 512], F32, name="ps_s")
                        for fc in range(FC):
                            nc.tensor.matmul(ps_s, (aTg[:, fc, jsl]),
                                             (BT_s[:, fc, tsl]),
                                             start=(fc == 0),
                                             stop=(fc == FC - 1))
                        nc.vector.tensor_add(ps_s, ps_s, negm_bc[:, tsl])
                        nc.scalar.activation(PT[:, tsl], ps_s,
                                             func=mybir.ActivationFunctionType.Exp)
                    pts.append(PT)

                for rc in range(RC):
                    ps_o = po_ps.tile([128, H], F32, name="ps_o")
                    rsl = slice(128 * rc, 128 * rc + 128)
                    for j in range(GRP):
                        nc.tensor.matmul(ps_o, (pts[j][:, rsl]), (abg[:, j, :]),
                                         start=(j == 0), stop=(j == GRP - 1))
                        # row-sum l via N=1 matmul, same stationary operand.
                        # One accumulation group for the whole bank: only the
                        # very first matmul clears has_written; each column's
                        # first touch then overwrites, later ones accumulate.
                        nc.tensor.matmul(psl[:, rc:rc + 1], (pts[j][:, rsl]),
                                         (ones_b),
                                         start=(g == 0 and rc == 0 and j == 0),
                                         stop=(g == NG - 1 and rc == RC - 1
                                               and j == GRP - 1))
                    if g == 0:
                        nc.vector.tensor_copy(out_acc[:, rc, :], ps_o)
                    else:
                        nc.vector.tensor_add(out_acc[:, rc, :],
                                             out_acc[:, rc, :], ps_o)

        # ----------------- epilogue -----------------
        with ExitStack() as ectx:
            ep = ectx.enter_context(tc.tile_pool(name="epil", bufs=4))
            ept_ps = ectx.enter_context(
                tc.tile_pool(name="ps_ept", bufs=2, space="PSUM"))
            epo_ps = ectx.enter_context(
                tc.tile_pool(name="ps_epo", bufs=2, space="PSUM"))
            rinv = ep.tile([128, RC], F32)
            nc.vector.reciprocal(rinv, psl)
            for rc in range(RC):
                rsl = slice(128 * rc, 128 * rc + 128)
                # transpose PA chunk: out_acc[:, rc, :] -> [f, r]
                ps_pt = ept_ps.tile([128, H], F32, name="ps_pt")
                for fc in range(FC):
                    fsl = slice(128 * fc, 128 * fc + 128)
                    nc.tensor.transpose(ps_pt[:, fsl], out_acc[:, rc, fsl],
                                        id_s)
                pat = ep.tile([128, FC, 128], F32R, name="pat")
                nc.scalar.copy(pat,
                               ps_pt.rearrange("p (c j) -> p c j", j=128))
                # att = (PA @ WvT) / l
                ps_att = epo_ps.tile([128, H], F32, name="ps_att")
                for fc in range(FC):
                    nc.tensor.matmul(ps_att, pat[:, fc, :], wvT_s[:, fc, :],
                                     start=(fc == 0), stop=(fc == FC - 1))
                att = ep.tile([128, H], F32, name="att")
                nc.vector.tensor_scalar_mul(att, ps_att, rinv[:, rc:rc + 1])
                nc.sync.dma_start(out=out_att[rsl, :], in_=att)
                ft = ep.tile([128, H], F32, name="ft")
                nc.vector.tensor_add(ft, att, a_loc_s[:, rc, :])
                nc.sync.dma_start(out=feat[rsl, :], in_=ft)

    nc.finalize()
    return nc


_NC_CACHE = []


def _get_nc():
    if not _NC_CACHE:
        _NC_CACHE.append(build())
    return _NC_CACHE[0]


def make_in_maps(inputs_a, inputs_v, W_q, W_k, W_v, W_m):
    a = np.ascontiguousarray(np.asarray(inputs_a, dtype=np.float32))
    v = np.ascontiguousarray(np.asarray(inputs_v, dtype=np.float32))
    wq = np.asarray(W_q, dtype=np.float32)
    wk = np.asarray(W_k, dtype=np.float32)
    wv = np.asarray(W_v, dtype=np.float32)
    wm = np.asarray(W_m, dtype=np.float32)
    aT = np.ascontiguousarray(a.T)
    ab = np.ascontiguousarray(a.astype(ml_dtypes.bfloat16))
    g1v = np.ascontiguousarray(wq.T @ wk)
    g2v = np.ascontiguousarray(wm.T @ wq)
    wvTv = np.ascontiguousarray(wv.T)
    in_maps = []
    for i in range(NCORE):
        sl = slice(R * i, R * (i + 1))
        in_maps.append({
            "aT": aT,
            "ab": ab,
            "aT_loc": np.ascontiguousarray(aT[:, sl]),
            "avT_loc": np.ascontiguousarray((a[sl] + v[sl]).T),
            "a_loc": np.ascontiguousarray(a[sl]),
            "g1": g1v,
            "g2": g2v,
            "wvT": wvTv,
        })
    return in_maps


def kernel(inputs_a, inputs_v, W_q, W_k, W_v, W_m, _run_kwargs=None):
    nc = _get_nc()
    in_maps = make_in_maps(inputs_a, inputs_v, W_q, W_k, W_v, W_m)
    res = run_bass_kernel_spmd(nc, in_maps, list(range(NCORE)),
                               **(_run_kwargs or {}))
    out_attention = np.concatenate(
        [res.results[i]["out_att"] for i in range(NCORE)], axis=0)
    feature_map = np.concatenate(
        [res.results[i]["feat"] for i in range(NCORE)], axis=0)
    kernel.last_results = res
    return (out_attention, feature_map)


# revision 13
# speedup vs baseline: 1.1073x; 1.0229x over previous
"""Trainium2 Bass kernel for the fused cross-attention layer.

Math restructuring (exact):
    S = Q_a K_a^T + (Q_a M_av^T)^T
      = a (W_q^T W_k) a^T + (a+v) (W_m^T W_q) a^T
      = B a^T,   B = a G + (a+v) G2,  G = W_q^T W_k,  G2 = W_m^T W_q
    alpha = softmax(S, axis=1);  out = alpha @ (a W_v^T);  feat = out + a

So the N x N score matrix is a single [N,H]x[H,N] matmul instead of two,
and the alpha_av^T "all-to-all coupling" disappears: each core only needs
its local rows of B plus the shared a^T stream.

Sharding: rows of the score matrix across 8 cores (1024 rows each).
Each core streams all 64 column-chunks of a^T (host-pretransposed, so
the PE does no transposes in the main loop), computes S^T (columns on
partitions) so the softmax-weighted PV matmul needs no P transposes,
and accumulates output in PSUM/SBUF.  G/G2/W_v^T are tiny host-side
weight preprocessing; P and the PV matmul run in bf16 (tolerance is
2e-2), with the softmax denominators produced by N=1 matmuls that share
the PV matmul's stationary operand.

Softmax shift: exact row-max is a partition-dim reduction in this layout,
but softmax only needs any per-row shift within ~+-80 of the true max.
We use m_r = 4.2*||B_r|| + 25 (validated: m - rowmax in [-14, +38] here,
overflow would need a >12-sigma row maximum).

All heavy fp32 matmuls run as float32r (FP22 mantissa truncation, full
PE rate); accumulation is always fp32 in PSUM.
"""

import sys

sys.path.insert(0, "/opt/trn_rl_repo")

from contextlib import ExitStack

import ml_dtypes
import numpy as np

import concourse.bacc as bacc
import concourse.bass as bass
import concourse.mybir as mybir
import concourse.tile as tile
from concourse.bass_utils import run_bass_kernel_spmd
from concourse.masks import make_identity

N, H, NCORE = 8192, 512, 8
R = N // NCORE          # 1024 rows per core
RC = R // 128           # 8 row chunks per core
FC = H // 128           # 4 feature chunks
CC = N // 128           # 64 column chunks (full N)
GRP = 8                 # column chunks per group
NG = CC // GRP

F32 = mybir.dt.float32
F32R = mybir.dt.float32r
BF16 = mybir.dt.bfloat16

MAX_SCALE = 4.2         # m_r = MAX_SCALE * ||B_r|| + MAX_BIAS
MAX_BIAS = 25.0


def build():
    nc = bacc.Bacc("TRN2", target_bir_lowering=False, debug=False,
                   num_devices=NCORE)
    # aT/ab are column/row-rotated per core so that the core's local row
    # block is group 0 of the stream: B^T computes straight from the
    # first prefetched group (softmax/PV are permutation-invariant sums
    # over the full column space).
    aT = nc.dram_tensor("aT", [H, N], F32, kind="ExternalInput").ap()
    ab = nc.dram_tensor("ab", [N, H], BF16, kind="ExternalInput").ap()
    avT_loc = nc.dram_tensor("avT_loc", [H, R], F32, kind="ExternalInput").ap()
    a_loc = nc.dram_tensor("a_loc", [R, H], F32, kind="ExternalInput").ap()
    g1 = nc.dram_tensor("g1", [H, H], F32, kind="ExternalInput").ap()
    g2 = nc.dram_tensor("g2", [H, H], F32, kind="ExternalInput").ap()
    wvT = nc.dram_tensor("wvT", [H, H], F32, kind="ExternalInput").ap()
    out_att = nc.dram_tensor("out_att", [R, H], F32, kind="ExternalOutput").ap()
    feat = nc.dram_tensor("feat", [R, H], F32, kind="ExternalOutput").ap()

    with tile.TileContext(nc) as tc, ExitStack() as ctx:
        persist = ctx.enter_context(tc.tile_pool(name="persist", bufs=1))
        id_s = persist.tile([128, 128], F32)
        make_identity(nc, id_s)
        ones_s = persist.tile([128, 1], F32R)
        ones_dram = nc.inline_tensor(np.ones((128, 1), np.float32), "ones_c")
        nc.sync.dma_start(out=ones_s, in_=ones_dram.ap().bitcast(F32R))
        ones_b = persist.tile([128, 1], BF16)
        nc.vector.memset(ones_b, 1.0)
        ones_row = persist.tile([1, 128], F32)
        nc.vector.memset(ones_row, 1.0)
        wvT_s = persist.tile([128, FC, H], F32R)    # W_v^T: [h, h']
        BT_s = persist.tile([128, FC, R], F32R)     # B^T local: [f, r]
        out_acc = persist.tile([128, RC, H], F32)   # PV accumulator
        negm_bc = persist.tile([128, R], F32)       # -m_r broadcast to 128 p
        a_loc_s = persist.tile([128, RC, H], F32)   # local a rows (epilogue)

        aT_r = aT.rearrange("(c p) n -> p c n", p=128).bitcast(F32R)
        ab_r = ab.rearrange("(c p) n -> p c n", p=128)
        ap_pool = ctx.enter_context(tc.tile_pool(name="atiles", bufs=2))

        # ----------------- setup -----------------
        with ExitStack() as sctx:
            sp = sctx.enter_context(tc.tile_pool(name="setup", bufs=1))

            # DMA issue order = criticality: the B^T matmuls need
            # aTg0 (local columns = group 0 of the rotated stream) + g1
            # first, then avTl+g2; epilogue-only tensors last.  aTg0
            # comes in halves so the first B^T tiles start sooner.
            aTg0 = ap_pool.tile([128, FC, 128 * GRP], F32R, name="aTg")
            nc.sync.dma_start(out=aTg0[:, :, 0:512], in_=aT_r[:, :, 0:512])
            g1_s = sp.tile([128, FC, H], F32R)
            nc.sync.dma_start(
                out=g1_s,
                in_=g1.rearrange("(c p) n -> p c n", p=128).bitcast(F32R))
            nc.sync.dma_start(out=aTg0[:, :, 512:1024],
                              in_=aT_r[:, :, 512:1024])
            g2_s = sp.tile([128, FC, H], F32R)
            nc.sync.dma_start(
                out=g2_s,
                in_=g2.rearrange("(c p) n -> p c n", p=128).bitcast(F32R))
            avTl = sp.tile([128, FC, R], F32R)
            nc.sync.dma_start(
                out=avTl,
                in_=avT_loc.rearrange("(c p) n -> p c n", p=128).bitcast(F32R))
            abg0 = ap_pool.tile([128, GRP, H], BF16, name="abg")
            nc.sync.dma_start(out=abg0, in_=ab_r[:, 0:GRP, :])
            nc.sync.dma_start(
                out=a_loc_s,
                in_=a_loc.rearrange("(c p) n -> p c n", p=128))
            nc.sync.dma_start(
                out=wvT_s,
                in_=wvT.rearrange("(c p) n -> p c n", p=128).bitcast(F32R))

            # B^T = G^T a^T + G2^T av^T  ([f, r], f on partitions),
            # two passes over 8 PSUM banks so the G pass starts as soon
            # as aTg0+g1 land (avTl+g2 are still in flight).  ||B_r||^2
            # squares read each finished PSUM tile directly.
            bt2 = sp.tile([128, FC, R], F32R)
            with ExitStack() as btctx:
                btp = btctx.enter_context(
                    tc.tile_pool(name="bt_ps", bufs=8, space="PSUM"))
                bt_ps = []
                for rt in range(R // 512):
                    for fc in range(FC):
                        ps_b = btp.tile([128, 512], F32, name="ps_bt")
                        msl = slice(128 * fc, 128 * fc + 128)
                        tsl = slice(512 * rt, 512 * rt + 512)
                        for kc in range(FC):
                            nc.tensor.matmul(ps_b, (g1_s[:, kc, msl]),
                                             (aTg0[:, kc, tsl]),
                                             start=(kc == 0), stop=False)
                        bt_ps.append(ps_b)
                for rt in range(R // 512):
                    for fc in range(FC):
                        ps_b = bt_ps[rt * FC + fc]
                        msl = slice(128 * fc, 128 * fc + 128)
                        tsl = slice(512 * rt, 512 * rt + 512)
                        for kc in range(FC):
                            nc.tensor.matmul(ps_b, (g2_s[:, kc, msl]),
                                             (avTl[:, kc, tsl]),
                                             start=False, stop=(kc == FC - 1))
                        nc.vector.tensor_copy(BT_s[:, fc, tsl], ps_b)
                        nc.scalar.square(bt2[:, fc, tsl], BT_s[:, fc, tsl])
            spp = sctx.enter_context(
                tc.tile_pool(name="setup_ps", bufs=2, space="PSUM"))

            # -m_r = -(MAX_SCALE * ||B_r|| + MAX_BIAS), broadcast to 128 p
            negm_f32 = sp.tile([1, R], F32)
            for rt in range(R // 512):
                tsl = slice(512 * rt, 512 * rt + 512)
                ps_n = spp.tile([1, 512], F32, name="ps_n")
                for fc in range(FC):
                    nc.tensor.matmul(ps_n, (ones_s), (bt2[:, fc, tsl]),
                                     start=(fc == 0), stop=(fc == FC - 1))
                nc.scalar.sqrt(negm_f32[0:1, tsl], ps_n)
            nc.vector.tensor_scalar(out=negm_f32, in0=negm_f32,
                                    scalar1=-MAX_SCALE, scalar2=-MAX_BIAS,
                                    op0=mybir.AluOpType.mult,
                                    op1=mybir.AluOpType.add)
            for rt in range(R // 512):
                tsl = slice(512 * rt, 512 * rt + 512)
                ps_bc = spp.tile([128, 512], F32, name="ps_gen")
                nc.tensor.matmul(ps_bc, ones_row, negm_f32[0:1, tsl],
                                 start=True, stop=True)
                nc.vector.tensor_copy(negm_bc[:, tsl], ps_bc)

        # ----------------- main sweep -----------------
        pl_ps = ctx.enter_context(
            tc.tile_pool(name="ps_l", bufs=1, space="PSUM"))
        psl = pl_ps.tile([128, RC], F32)            # row sums l, [r, rc]
        with ExitStack() as mctx:
            pp = mctx.enter_context(tc.tile_pool(name="ptiles", bufs=GRP + 2))
            ps_ps = mctx.enter_context(
                tc.tile_pool(name="ps_s", bufs=4, space="PSUM"))
            po_ps = mctx.enter_context(
                tc.tile_pool(name="ps_o", bufs=2, space="PSUM"))
            for g in range(NG):
                if g == 0:
                    aTg, abg = aTg0, abg0
                else:
                    csl = slice(128 * GRP * g, 128 * GRP * (g + 1))
                    aTg = ap_pool.tile([128, FC, 128 * GRP], F32R, name="aTg")
                    nc.sync.dma_start(out=aTg, in_=aT_r[:, :, csl])
                    abg = ap_pool.tile([128, GRP, H], BF16, name="abg")
                    nc.sync.dma_start(
                        out=abg, in_=ab_r[:, GRP * g:GRP * (g + 1), :])

                pts = []
                for j in range(GRP):
                    jsl = slice(128 * j, 128 * j + 128)
                    PT = pp.tile([128, R], BF16, name="PT")
                    for rt in range(R // 512):
                        tsl = slice(512 * rt, 512 * rt + 512)
                        ps_s = ps_ps.tile([128, 512], F32, name="ps_s")
                        for fc in range(FC):
                            nc.tensor.matmul(ps_s, (aTg[:, fc, jsl]),
                                             (BT_s[:, fc, tsl]),
                                             start=(fc == 0),
                                             stop=(fc == FC - 1))
                        nc.vector.tensor_add(ps_s, ps_s, negm_bc[:, tsl])
                        nc.scalar.activation(PT[:, tsl], ps_s,
                                             func=mybir.ActivationFunctionType.Exp)
                    pts.append(PT)

                for rc in range(RC):
                    ps_o = po_ps.tile([128, H], F32, name="ps_o")
                    rsl = slice(128 * rc, 128 * rc + 128)
                    for j in range(GRP):
                        nc.tensor.matmul(ps_o, (pts[j][:, rsl]), (abg[:, j, :]),
                                         start=(j == 0), stop=(j == GRP - 1))
                        # row-sum l via N=1 matmul, same stationary operand.
                        # One accumulation group for the whole bank: only the
                        # very first matmul clears has_written; each column's
                        # first touch then overwrites, later ones accumulate.
                        nc.tensor.matmul(psl[:, rc:rc + 1], (pts[j][:, rsl]),
                                         (ones_b),
                                         start=(g == 0 and rc == 0 and j == 0),
                                         stop=(g == NG - 1 and rc == RC - 1
                                               and j == GRP - 1))
                    if g == 0:
                        nc.vector.tensor_copy(out_acc[:, rc, :], ps_o)
                    else:
                        nc.vector.tensor_add(out_acc[:, rc, :],
                                             out_acc[:, rc, :], ps_o)

        # ----------------- epilogue -----------------
        with ExitStack() as ectx:
            ep = ectx.enter_context(tc.tile_pool(name="epil", bufs=4))
            ept_ps = ectx.enter_context(
                tc.tile_pool(name="ps_ept", bufs=2, space="PSUM"))
            epo_ps = ectx.enter_context(
                tc.tile_pool(name="ps_epo", bufs=2, space="PSUM"))
            rinv = ep.tile([128, RC], F32)
            nc.vector.reciprocal(rinv, psl)
            for rc in range(RC):
                rsl = slice(128 * rc, 128 * rc + 128)
                # transpose PA chunk: out_acc[:, rc, :] -> [f, r]
                ps_pt = ept_ps.tile([128, H], F32, name="ps_pt")
                for fc in range(FC):
                    fsl = slice(128 * fc, 128 * fc + 128)
                    nc.tensor.transpose(ps_pt[:, fsl], out_acc[:, rc, fsl],
                                        id_s)
                pat = ep.tile([128, FC, 128], F32R, name="pat")
                nc.scalar.copy(pat,
                               ps_pt.rearrange("p (c j) -> p c j", j=128))
                # att = (PA @ WvT) / l
                ps_att = epo_ps.tile([128, H], F32, name="ps_att")
                for fc in range(FC):
                    nc.tensor.matmul(ps_att, pat[:, fc, :], wvT_s[:, fc, :],
                                     start=(fc == 0), stop=(fc == FC - 1))
                att = ep.tile([128, H], F32, name="att")
                nc.vector.tensor_scalar_mul(att, ps_att, rinv[:, rc:rc + 1])
                nc.sync.dma_start(out=out_att[rsl, :], in_=att)
                ft = ep.tile([128, H], F32, name="ft")
                nc.vector.tensor_add(ft, att, a_loc_s[:, rc, :])
                nc.sync.dma_start(out=feat[rsl, :], in_=ft)

    nc.finalize()
    return nc


_NC_CACHE = []


def _get_nc():
    if not _NC_CACHE:
        _NC_CACHE.append(build())
    return _NC_CACHE[0]


def make_in_maps(inputs_a, inputs_v, W_q, W_k, W_v, W_m):
    a = np.ascontiguousarray(np.asarray(inputs_a, dtype=np.float32))
    v = np.ascontiguousarray(np.asarray(inputs_v, dtype=np.float32))
    wq = np.asarray(W_q, dtype=np.float32)
    wk = np.asarray(W_k, dtype=np.float32)
    wv = np.asarray(W_v, dtype=np.float32)
    wm = np.asarray(W_m, dtype=np.float32)
    aT = np.ascontiguousarray(a.T)
    ab = np.ascontiguousarray(a.astype(ml_dtypes.bfloat16))
    g1v = np.ascontiguousarray(wq.T @ wk)
    g2v = np.ascontiguousarray(wm.T @ wq)
    wvTv = np.ascontiguousarray(wv.T)
    in_maps = []
    for i in range(NCORE):
        sl = slice(R * i, R * (i + 1))
        # rotate the shared stream so core i's local rows come first
        in_maps.append({
            "aT": np.ascontiguousarray(np.roll(aT, -R * i, axis=1)),
            "ab": np.ascontiguousarray(np.roll(ab, -R * i, axis=0)),
            "avT_loc": np.ascontiguousarray((a[sl] + v[sl]).T),
            "a_loc": np.ascontiguousarray(a[sl]),
            "g1": g1v,
            "g2": g2v,
            "wvT": wvTv,
        })
    return in_maps


def kernel(inputs_a, inputs_v, W_q, W_k, W_v, W_m, _run_kwargs=None):
    nc = _get_nc()
    in_maps = make_in_maps(inputs_a, inputs_v, W_q, W_k, W_v, W_m)
    res = run_bass_kernel_spmd(nc, in_maps, list(range(NCORE)),
                               **(_run_kwargs or {}))
    out_attention = np.concatenate(
        [res.results[i]["out_att"] for i in range(NCORE)], axis=0)
    feature_map = np.concatenate(
        [res.results[i]["feat"] for i in range(NCORE)], axis=0)
    kernel.last_results = res
    return (out_attention, feature_map)


# revision 25
# speedup vs baseline: 1.1486x; 1.0373x over previous
"""Trainium2 Bass kernel for the fused cross-attention layer.

Math restructuring (exact):
    S = Q_a K_a^T + (Q_a M_av^T)^T
      = a (W_q^T W_k) a^T + (a+v) (W_m^T W_q) a^T
      = B a^T,   B = a G + (a+v) G2,  G = W_q^T W_k,  G2 = W_m^T W_q
    alpha = softmax(S, axis=1);  out = alpha @ (a W_v^T);  feat = out + a

So the N x N score matrix is a single [N,H]x[H,N] matmul instead of two,
and the alpha_av^T "all-to-all coupling" disappears: each core only needs
its local rows of B plus the shared a^T stream.

Sharding: rows of the score matrix across 8 cores (1024 rows each).
Each core streams all 64 column-chunks of a^T (host-pretransposed, so
the PE does no transposes in the main loop), computes S^T (columns on
partitions) so the softmax-weighted PV matmul needs no P transposes,
and accumulates output in PSUM/SBUF.  G/G2/W_v^T are tiny host-side
weight preprocessing; P and the PV matmul run in bf16 (tolerance is
2e-2), with the softmax denominators produced by N=1 matmuls that share
the PV matmul's stationary operand.

Softmax shift: exact row-max is a partition-dim reduction in this layout,
but softmax only needs any per-row shift within ~+-80 of the true max.
We use m_r = 4.2*||B_r|| + 25 (validated: m - rowmax in [-14, +38] here,
overflow would need a >12-sigma row maximum).

All heavy fp32 matmuls run as float32r (FP22 mantissa truncation, full
PE rate); accumulation is always fp32 in PSUM.
"""

import sys

sys.path.insert(0, "/opt/trn_rl_repo")

from contextlib import ExitStack

import ml_dtypes
import numpy as np

import concourse.bacc as bacc
import concourse.bass as bass
import concourse.mybir as mybir
import concourse.tile as tile
from concourse.bass_utils import run_bass_kernel_spmd
from concourse.masks import make_identity

N, H, NCORE = 8192, 512, 8
R = N // NCORE          # 1024 rows per core
RC = R // 128           # 8 row chunks per core
FC = H // 128           # 4 feature chunks
CC = N // 128           # 64 column chunks (full N)
GRP = 8                 # column chunks per group
NG = CC // GRP

F32 = mybir.dt.float32
F32R = mybir.dt.float32r
BF16 = mybir.dt.bfloat16

MAX_SCALE = 4.2         # m_r = MAX_SCALE * ||B_r|| + MAX_BIAS
MAX_BIAS = 25.0


def build():
    nc = bacc.Bacc("TRN2", target_bir_lowering=False, debug=False,
                   num_devices=NCORE)
    # aT/ab are column/row-rotated per core so that the core's local row
    # block is group 0 of the stream: B^T computes straight from the
    # first prefetched group (softmax/PV are permutation-invariant sums
    # over the full column space).
    aT = nc.dram_tensor("aT", [H, N], F32, kind="ExternalInput").ap()
    ab = nc.dram_tensor("ab", [N, H], BF16, kind="ExternalInput").ap()
    avT_loc = nc.dram_tensor("avT_loc", [H, R], F32, kind="ExternalInput").ap()
    a_loc = nc.dram_tensor("a_loc", [R, H], F32, kind="ExternalInput").ap()
    g1 = nc.dram_tensor("g1", [H, H], F32, kind="ExternalInput").ap()
    g2 = nc.dram_tensor("g2", [H, H], F32, kind="ExternalInput").ap()
    wvT = nc.dram_tensor("wvT", [H, H], F32, kind="ExternalInput").ap()
    out_att = nc.dram_tensor("out_att", [R, H], F32, kind="ExternalOutput").ap()
    feat = nc.dram_tensor("feat", [R, H], F32, kind="ExternalOutput").ap()

    with tile.TileContext(nc) as tc, ExitStack() as ctx:
        persist = ctx.enter_context(tc.tile_pool(name="persist", bufs=1))
        id_s = persist.tile([128, 128], F32)
        make_identity(nc, id_s)
        ones_s = persist.tile([128, 1], F32R)
        ones_dram = nc.inline_tensor(np.ones((128, 1), np.float32), "ones_c")
        nc.sync.dma_start(out=ones_s, in_=ones_dram.ap().bitcast(F32R))
        ones_b = persist.tile([128, 1], BF16)
        nc.vector.memset(ones_b, 1.0)
        ones_row = persist.tile([1, 128], F32)
        nc.vector.memset(ones_row, 1.0)
        nrow = persist.tile([1, 128], F32)          # -MAX_SCALE row for bcast
        nc.vector.memset(nrow, -MAX_SCALE)
        nbias = persist.tile([128, 1], F32)         # -MAX_BIAS for exp bias
        nc.vector.memset(nbias, -MAX_BIAS)
        wvT_s = persist.tile([128, FC, H], F32R)    # W_v^T: [h, h']
        BT_s = persist.tile([128, FC, R], F32R)     # B^T local: [f, r]
        out_acc = persist.tile([128, RC, H], F32)   # PV accumulator
        negm_bc = persist.tile([128, R], F32)       # -m_r broadcast to 128 p
        a_loc_s = persist.tile([128, RC, H], F32)   # local a rows (epilogue)

        aT_r = aT.rearrange("(c p) n -> p c n", p=128).bitcast(F32R)
        ab_r = ab.rearrange("(c p) n -> p c n", p=128)
        ap_pool = ctx.enter_context(tc.tile_pool(name="atiles", bufs=2))

        # ----------------- setup -----------------
        with ExitStack() as sctx:
            sp = sctx.enter_context(tc.tile_pool(name="setup", bufs=1))

            # DMA issue order = criticality: the B^T matmuls need
            # aTg0 (local columns = group 0 of the rotated stream) + g1
            # first, then avTl+g2; epilogue-only tensors last.  aTg0
            # comes in halves so the first B^T tiles start sooner.
            aTg0 = ap_pool.tile([128, FC, 128 * GRP], F32R, name="aTg")
            nc.sync.dma_start(out=aTg0[:, :, 0:512], in_=aT_r[:, :, 0:512])
            g1_s = sp.tile([128, FC, H], F32R)
            nc.sync.dma_start(
                out=g1_s,
                in_=g1.rearrange("(c p) n -> p c n", p=128).bitcast(F32R))
            nc.sync.dma_start(out=aTg0[:, :, 512:1024],
                              in_=aT_r[:, :, 512:1024])
            g2_s = sp.tile([128, FC, H], F32R)
            nc.sync.dma_start(
                out=g2_s,
                in_=g2.rearrange("(c p) n -> p c n", p=128).bitcast(F32R))
            avTl = sp.tile([128, FC, R], F32R)
            avT_r = avT_loc.rearrange("(c p) n -> p c n", p=128).bitcast(F32R)
            nc.sync.dma_start(out=avTl[:, :, 0:512], in_=avT_r[:, :, 0:512])
            nc.sync.dma_start(out=avTl[:, :, 512:1024],
                              in_=avT_r[:, :, 512:1024])
            abg0 = ap_pool.tile([128, GRP, H], BF16, name="abg")
            nc.sync.dma_start(out=abg0, in_=ab_r[:, 0:GRP, :])
            nc.sync.dma_start(
                out=a_loc_s,
                in_=a_loc.rearrange("(c p) n -> p c n", p=128))
            nc.sync.dma_start(
                out=wvT_s,
                in_=wvT.rearrange("(c p) n -> p c n", p=128).bitcast(F32R))

            # B^T = G^T a^T + G2^T av^T  ([f, r], f on partitions),
            # two passes over 8 PSUM banks so the G pass starts as soon
            # as aTg0+g1 land (avTl+g2 are still in flight).  ||B_r||^2
            # squares read each finished PSUM tile directly.
            bt2 = sp.tile([128, FC, R], F32R)
            with ExitStack() as btctx:
                btp = btctx.enter_context(
                    tc.tile_pool(name="bt_ps", bufs=8, space="PSUM"))
                bt_ps = []
                for rt in range(R // 512):
                    for fc in range(FC):
                        ps_b = btp.tile([128, 512], F32, name="ps_bt")
                        msl = slice(128 * fc, 128 * fc + 128)
                        tsl = slice(512 * rt, 512 * rt + 512)
                        for kc in range(FC):
                            nc.tensor.matmul(ps_b, (g1_s[:, kc, msl]),
                                             (aTg0[:, kc, tsl]),
                                             start=(kc == 0), stop=False)
                        bt_ps.append(ps_b)
                for rt in range(R // 512):
                    for fc in range(FC):
                        ps_b = bt_ps[rt * FC + fc]
                        msl = slice(128 * fc, 128 * fc + 128)
                        tsl = slice(512 * rt, 512 * rt + 512)
                        for kc in range(FC):
                            nc.tensor.matmul(ps_b, (g2_s[:, kc, msl]),
                                             (avTl[:, kc, tsl]),
                                             start=False, stop=(kc == FC - 1))
                        nc.vector.tensor_copy(BT_s[:, fc, tsl], ps_b)
                        nc.scalar.square(bt2[:, fc, tsl], BT_s[:, fc, tsl])
            spp = sctx.enter_context(
                tc.tile_pool(name="setup_ps", bufs=2, space="PSUM"))

            # negm_bc = -MAX_SCALE * ||B_r||, broadcast to 128 partitions
            # (the -MAX_BIAS shift is folded into the exp's bias operand)
            negm_f32 = sp.tile([1, R], F32)
            for rt in range(R // 512):
                tsl = slice(512 * rt, 512 * rt + 512)
                ps_n = spp.tile([1, 512], F32, name="ps_n")
                for fc in range(FC):
                    nc.tensor.matmul(ps_n, (ones_s), (bt2[:, fc, tsl]),
                                     start=(fc == 0), stop=(fc == FC - 1))
                nc.scalar.sqrt(negm_f32[0:1, tsl], ps_n)
            for rt in range(R // 512):
                tsl = slice(512 * rt, 512 * rt + 512)
                ps_bc = spp.tile([128, 512], F32, name="ps_gen")
                nc.tensor.matmul(ps_bc, nrow, negm_f32[0:1, tsl],
                                 start=True, stop=True)
                nc.vector.tensor_copy(negm_bc[:, tsl], ps_bc)

        # ----------------- main sweep -----------------
        pl_ps = ctx.enter_context(
            tc.tile_pool(name="ps_l", bufs=1, space="PSUM"))
        psl = pl_ps.tile([128, RC], F32)            # row sums l, [r, rc]
        with ExitStack() as mctx:
            pp = mctx.enter_context(tc.tile_pool(name="ptiles", bufs=GRP + 2))
            lsum = mctx.enter_context(tc.tile_pool(name="lsum", bufs=2))
            ps_ps = mctx.enter_context(
                tc.tile_pool(name="ps_s", bufs=4, space="PSUM"))
            po_ps = mctx.enter_context(
                tc.tile_pool(name="ps_o", bufs=2, space="PSUM"))
            for g in range(NG):
                if g == 0:
                    aTg, abg = aTg0, abg0
                else:
                    csl = slice(128 * GRP * g, 128 * GRP * (g + 1))
                    aTg = ap_pool.tile([128, FC, 128 * GRP], F32R, name="aTg")
                    nc.sync.dma_start(out=aTg, in_=aT_r[:, :, csl])
                    abg = ap_pool.tile([128, GRP, H], BF16, name="abg")
                    nc.sync.dma_start(
                        out=abg, in_=ab_r[:, GRP * g:GRP * (g + 1), :])

                pts = []
                for j in range(GRP):
                    jsl = slice(128 * j, 128 * j + 128)
                    PT = pp.tile([128, R], BF16, name="PT")
                    for rt in range(R // 512):
                        tsl = slice(512 * rt, 512 * rt + 512)
                        ps_s = ps_ps.tile([128, 512], F32, name="ps_s")
                        for fc in range(FC):
                            nc.tensor.matmul(ps_s, (aTg[:, fc, jsl]),
                                             (BT_s[:, fc, tsl]),
                                             start=(fc == 0),
                                             stop=(fc == FC - 1))
                        nc.vector.tensor_add(ps_s, ps_s, negm_bc[:, tsl])
                        nc.scalar.activation(PT[:, tsl], ps_s,
                                             func=mybir.ActivationFunctionType.Exp,
                                             bias=nbias)
                    pts.append(PT)

                # group-level P column-sum tree on the DVE: one [128, R]
                # partial per group, so l needs only 8 N=1 matmuls/group
                s2 = [lsum.tile([128, R], F32R, name=f"s2_{k}")
                      for k in range(2)]
                s4 = lsum.tile([128, R], F32R, name="s4")
                ptsum = lsum.tile([128, R], BF16, name="ptsum")
                nc.vector.tensor_add(s2[0], pts[0], pts[1])
                nc.vector.tensor_add(s2[1], pts[2], pts[3])
                nc.vector.tensor_add(s4, s2[0], s2[1])
                nc.vector.tensor_add(s2[0], pts[4], pts[5])
                nc.vector.tensor_add(s2[1], pts[6], pts[7])
                nc.vector.tensor_add(s2[0], s2[0], s2[1])
                nc.vector.tensor_add(ptsum, s4, s2[0])

                for rc in range(RC):
                    ps_o = po_ps.tile([128, H], F32, name="ps_o")
                    rsl = slice(128 * rc, 128 * rc + 128)
                    for j in range(GRP):
                        nc.tensor.matmul(ps_o, (pts[j][:, rsl]), (abg[:, j, :]),
                                         start=(j == 0), stop=(j == GRP - 1))
                    if g == 0:
                        nc.vector.tensor_copy(out_acc[:, rc, :], ps_o)
                    else:
                        nc.vector.tensor_add(out_acc[:, rc, :],
                                             out_acc[:, rc, :], ps_o)
                for rc in range(RC):
                    # row-sum l via N=1 matmul.  One accumulation group for
                    # the whole psl bank: only the very first matmul clears
                    # has_written; each column's first touch then overwrites,
                    # later ones accumulate.
                    rsl = slice(128 * rc, 128 * rc + 128)
                    nc.tensor.matmul(psl[:, rc:rc + 1], (ptsum[:, rsl]),
                                     (ones_b),
                                     start=(g == 0 and rc == 0),
                                     stop=(g == NG - 1 and rc == RC - 1))

        # ----------------- epilogue -----------------
        with ExitStack() as ectx:
            ep = ectx.enter_context(tc.tile_pool(name="epil", bufs=4))
            ept_ps = ectx.enter_context(
                tc.tile_pool(name="ps_ept", bufs=3, space="PSUM"))
            epo_ps = ectx.enter_context(
                tc.tile_pool(name="ps_epo", bufs=3, space="PSUM"))
            rinv = ep.tile([128, RC], F32)
            nc.vector.reciprocal(rinv, psl)
            for rc in range(RC):
                rsl = slice(128 * rc, 128 * rc + 128)
                # transpose PA chunk: out_acc[:, rc, :] -> [f, r]
                ps_pt = ept_ps.tile([128, H], F32, name="ps_pt")
                for fc in range(FC):
                    fsl = slice(128 * fc, 128 * fc + 128)
                    nc.tensor.transpose(ps_pt[:, fsl], out_acc[:, rc, fsl],
                                        id_s)
                pat = ep.tile([128, FC, 128], F32R, name="pat")
                nc.scalar.copy(pat,
                               ps_pt.rearrange("p (c j) -> p c j", j=128))
                # att = (PA @ WvT) / l
                ps_att = epo_ps.tile([128, H], F32, name="ps_att")
                for fc in range(FC):
                    nc.tensor.matmul(ps_att, pat[:, fc, :], wvT_s[:, fc, :],
                                     start=(fc == 0), stop=(fc == FC - 1))
                att = ep.tile([128, H], F32, name="att")
                nc.vector.tensor_scalar_mul(att, ps_att, rinv[:, rc:rc + 1])
                nc.sync.dma_start(out=out_att[rsl, :], in_=att)
                ft = ep.tile([128, H], F32, name="ft")
                nc.vector.tensor_add(ft, att, a_loc_s[:, rc, :])
                nc.sync.dma_start(out=feat[rsl, :], in_=ft)

    nc.finalize()
    return nc


_NC_CACHE = []


def _get_nc():
    if not _NC_CACHE:
        _NC_CACHE.append(build())
    return _NC_CACHE[0]


def make_in_maps(inputs_a, inputs_v, W_q, W_k, W_v, W_m):
    a = np.ascontiguousarray(np.asarray(inputs_a, dtype=np.float32))
    v = np.ascontiguousarray(np.asarray(inputs_v, dtype=np.float32))
    wq = np.asarray(W_q, dtype=np.float32)
    wk = np.asarray(W_k, dtype=np.float32)
    wv = np.asarray(W_v, dtype=np.float32)
    wm = np.asarray(W_m, dtype=np.float32)
    aT = np.ascontiguousarray(a.T)
    ab = np.ascontiguousarray(a.astype(ml_dtypes.bfloat16))
    g1v = np.ascontiguousarray(wq.T @ wk)
    g2v = np.ascontiguousarray(wm.T @ wq)
    wvTv = np.ascontiguousarray(wv.T)
    in_maps = []
    for i in range(NCORE):
        sl = slice(R * i, R * (i + 1))
        # rotate the shared stream so core i's local rows come first
        in_maps.append({
            "aT": np.ascontiguousarray(np.roll(aT, -R * i, axis=1)),
            "ab": np.ascontiguousarray(np.roll(ab, -R * i, axis=0)),
            "avT_loc": np.ascontiguousarray((a[sl] + v[sl]).T),
            "a_loc": np.ascontiguousarray(a[sl]),
            "g1": g1v,
            "g2": g2v,
            "wvT": wvTv,
        })
    return in_maps


def kernel(inputs_a, inputs_v, W_q, W_k, W_v, W_m, _run_kwargs=None):
    nc = _get_nc()
    in_maps = make_in_maps(inputs_a, inputs_v, W_q, W_k, W_v, W_m)
    res = run_bass_kernel_spmd(nc, in_maps, list(range(NCORE)),
                               **(_run_kwargs or {}))
    out_attention = np.concatenate(
        [res.results[i]["out_att"] for i in range(NCORE)], axis=0)
    feature_map = np.concatenate(
        [res.results[i]["feat"] for i in range(NCORE)], axis=0)
    kernel.last_results = res
    return (out_attention, feature_map)
